# revision 1
# baseline (speedup 1.0000x reference)
"""Trainium2 Bass kernel for nn_AdapterController (moe_routing).

Math (per sentence):
  z = LayerNorm(x) * g + b                      [S, D]
  probs = softmax(BN(mean_s z) @ Wr + br)       [E]
  idx = argmax(probs); gate = probs[idx]
  y = (relu(z @ W_down[idx] + b_down[idx]) @ W_up[idx] + b_up[idx]) * gate

Strategy: data-parallel over batch (8 sentences per core, no collectives).

V3 design (no on-device transpose, minimal PE work, gathered experts):
  - Host ships x twice in bf16: token-major xa (for LN stats only) and
    d-major xt (host-pre-transposed; feeds both router and mm1).
  - LN is FUSED into mm1: h = rs_t*(x @ Wd) - (mu_t*rs_t)*colsum(Wd) + bd,
    applied on the small [H, S] output, so the normalize pass over [S, D]
    and the z-transpose disappear entirely.
  - Router: w[d] = sum_t x[d,t]*rs[t] via DVE tensor_tensor_reduce against
    a GpSimd-broadcast rs row; logits assembled per-partition and combined
    with GpSimd partition_all_reduce, so softmax/argmax/gate run replicated
    on all 128 partitions.
  - Expert selection: data-dependent dma_gather (SWDGE) of the selected
    expert's W_down/W_up rows from HBM using on-device computed int16
    indices; gate is applied in the y PSUM->SBUF copy on ACT.
  - mm1/mm2 are the only heavy PE work; y is stored as bf16 (host upcasts).
"""

import sys

if "/opt/trn_rl_repo" not in sys.path:
    sys.path.insert(0, "/opt/trn_rl_repo")

from contextlib import ExitStack

import ml_dtypes
import numpy as np

import concourse.bacc as bacc
import concourse.bass as bass
import concourse.bass_isa as bass_isa
import concourse.tile as tile
from concourse import mybir
from concourse.bass_utils import run_bass_kernel_spmd

B, S, D, H, E = 64, 1024, 1024, 64, 8
NCORES = 8
BLOC = B // NCORES
P = 128
TC = S // P  # token chunks per sentence
DC = D // P  # d chunks
HP = H + 1  # h rows plus the ones row for the bias matmul
NWU = 80  # wu gather indices padded to a multiple of 16
EPS = 1e-5
FP32 = mybir.dt.float32
BF16 = mybir.dt.bfloat16
I16 = mybir.dt.int16

_CACHE = {}

AL = mybir.AluOpType


def _build_kernel():
    nc = bacc.Bacc(
        "TRN2",
        target_bir_lowering=False,
        debug=False,
        enable_asserts=False,
        num_devices=NCORES,
    )
    xa_ext = nc.dram_tensor("xa", [BLOC, S, D], BF16, kind="ExternalInput").ap()
    xt_ext = nc.dram_tensor("xt", [BLOC, DC, P, S], BF16, kind="ExternalInput").ap()
    wr_ext = nc.dram_tensor("wr", [DC, P, 2 * E], BF16, kind="ExternalInput").ap()
    ucbc_ext = nc.dram_tensor("ucbc", [P, 2, E], FP32, kind="ExternalInput").ap()
    wdg_ext = nc.dram_tensor("wdg", [E * P, DC * H], BF16, kind="ExternalInput").ap()
    wug_ext = nc.dram_tensor("wug", [E * NWU, D], BF16, kind="ExternalInput").ap()
    bdncs_ext = nc.dram_tensor("bdncs", [E * H, H], FP32, kind="ExternalInput").ap()
    ident_ext = nc.dram_tensor("ident", [P, P], FP32, kind="ExternalInput").ap()
    iota_ext = nc.dram_tensor("iota", [P, E], FP32, kind="ExternalInput").ap()
    wdb_ext = nc.dram_tensor("wdb", [P, P // 16], I16, kind="ExternalInput").ap()
    hb_ext = nc.dram_tensor("hb", [P, H // 16], I16, kind="ExternalInput").ap()
    wub_ext = nc.dram_tensor("wub", [P, NWU // 16], I16, kind="ExternalInput").ap()
    out_ext = nc.dram_tensor("out", [BLOC, S, D], BF16, kind="ExternalOutput").ap()

    with tile.TileContext(nc) as tc:
        _body(tc, out_ext, xa_ext, xt_ext, wr_ext, ucbc_ext, wdg_ext, wug_ext,
              bdncs_ext, ident_ext, iota_ext, wdb_ext, wub_ext, hb_ext)

    nc.compile()
    return nc


def _body(tc, out_ext, xa_ext, xt_ext, wr_ext, ucbc_ext, wdg_ext, wug_ext,
          bdncs_ext, ident_ext, iota_ext, wdb_ext, wub_ext, hb_ext):
    nc = tc.nc
    with ExitStack() as ctx:
        consts = ctx.enter_context(tc.tile_pool(name="consts", bufs=1))
        xa_pool = ctx.enter_context(tc.tile_pool(name="xap", bufs=2))
        xt_pool = ctx.enter_context(tc.tile_pool(name="xtp", bufs=3))
        y_pool = ctx.enter_context(tc.tile_pool(name="yp", bufs=2))
        st_pool = ctx.enter_context(tc.tile_pool(name="stp", bufs=3))
        rb_pool = ctx.enter_context(tc.tile_pool(name="rbp", bufs=3))
        w_pool = ctx.enter_context(tc.tile_pool(name="wp", bufs=3))
        h_pool = ctx.enter_context(tc.tile_pool(name="hp", bufs=3))
        e_pool = ctx.enter_context(tc.tile_pool(name="ep", bufs=3))
        junk_pool = ctx.enter_context(tc.tile_pool(name="junk", bufs=1))
        pp_h = ctx.enter_context(tc.tile_pool(name="pph", bufs=2, space="PSUM"))
        pp_y = ctx.enter_context(tc.tile_pool(name="ppy", bufs=2, space="PSUM"))
        pp_m = ctx.enter_context(tc.tile_pool(name="ppm", bufs=1, space="PSUM"))
        pp_xr = ctx.enter_context(tc.tile_pool(name="ppxr", bufs=2, space="PSUM"))

        # constants
        ident_sb = consts.tile([P, P], FP32)
        nc.scalar.dma_start(ident_sb, ident_ext)
        wr_sb = consts.tile([P, DC, 2 * E], BF16)
        nc.scalar.dma_start(wr_sb, wr_ext.rearrange("dc p e -> p dc e"))
        ucbc = consts.tile([P, 2, E], FP32)
        nc.scalar.dma_start(ucbc, ucbc_ext)

        iota_sb = consts.tile([P, E], FP32)
        nc.scalar.dma_start(iota_sb, iota_ext)
        wdb_sb = consts.tile([P, P // 16], I16)
        nc.scalar.dma_start(wdb_sb, wdb_ext)
        hb_sb = consts.tile([P, H // 16], I16)
        nc.scalar.dma_start(hb_sb, hb_ext)
        wub_sb = consts.tile([P, NWU // 16], I16)
        nc.scalar.dma_start(wub_sb, wub_ext)
        junk = junk_pool.tile([P, S], BF16)
        junk8 = junk_pool.tile([P, DC], FP32)
        junk_xr = junk_pool.tile([2 * E, S], FP32)

        state = {}

        def front(b):
            # ---- loads ----
            xa = xa_pool.tile([P, TC, D], BF16, tag="xa")
            nc.scalar.dma_start(xa, xa_ext[b].rearrange("(tc p) d -> p tc d", p=P))
            xt = xt_pool.tile([P, DC, S], BF16, tag="xt")
            nc.scalar.dma_start(xt, xt_ext[b].rearrange("dc p s -> p dc s"))

            # ---- layernorm stats ----
            mv = st_pool.tile([P, TC, 2], FP32, tag="mv")
            for t in range(TC):
                stats = st_pool.tile([P, 2, 6], FP32, tag="bnstats")
                xv = xa[:, t].rearrange("p (g f) -> p g f", g=2)
                nc.vector.bn_stats(stats[:, 0], xv[:, 0])
                nc.vector.bn_stats(stats[:, 1], xv[:, 1])
                nc.vector.bn_aggr(mv[:, t], stats)
            # rs = rsqrt(var + eps): linear seed + 3 Newton iterations (DVE)
            rsmrs = st_pool.tile([P, 2 * TC], FP32, tag="rsmrs")
            rs = rsmrs[:, 0:TC]
            mrs = rsmrs[:, TC : 2 * TC]
            vv = st_pool.tile([P, TC], FP32, tag="vv")
            nc.vector.tensor_scalar_add(vv, mv[:, :, 1], float(EPS))
            rcp = st_pool.tile([P, TC], FP32, tag="rcp")
            nc.vector.reciprocal(rcp, vv)
            nc.scalar.activation(rs, rcp, mybir.ActivationFunctionType.Sqrt)
            nc.vector.tensor_mul(mrs, mv[:, :, 0], rs)

            # ---- transpose rs/mrs to rows, broadcast across partitions ----
            ps_t = pp_m.tile([2 * TC, P], FP32, tag="pst")
            nc.tensor.transpose(ps_t, rsmrs, ident_sb)
            rmT = st_pool.tile([2 * TC, P], BF16, tag="rmT")
            nc.scalar.copy(rmT, ps_t)
            rs_row = st_pool.tile([1, S], BF16, tag="rs_row")
            mrs_row = st_pool.tile([1, S], BF16, tag="mrs_row")
            nc.sync.dma_start(
                rs_row.rearrange("a (t p) -> a t p", p=P), rmT[0:TC]
            )
            nc.sync.dma_start(
                mrs_row.rearrange("a (t p) -> a t p", p=P), rmT[TC : 2 * TC]
            )
            rs_b = rb_pool.tile([P, S], BF16, tag="rsb")
            mrs_b = rb_pool.tile([H, S], BF16, tag="mrsb")
            nc.gpsimd.partition_broadcast(rs_b, rs_row)
            nc.gpsimd.partition_broadcast(mrs_b, mrs_row)

            # ---- router: xrT[e, t] = sum_d wr[d, e] * x[d, t] on PE ----
            # wr is split hi/lo bf16 (exact), so M = 2E
            xrT = st_pool.tile([2 * E, S], FP32, tag="xrT")
            for half in range(2):
                sl = slice(half * 512, (half + 1) * 512)
                ps_xr = pp_xr.tile([2 * E, 512], FP32, tag="psxr")
                for dc in range(DC):
                    nc.tensor.matmul(
                        ps_xr, wr_sb[:, dc], xt[:, dc, sl],
                        start=(dc == 0), stop=(dc == DC - 1),
                    )
                nc.scalar.copy(xrT[:, sl], ps_xr)
            # logits_raw[e] = sum_t xrT[e, t] * rs[t]  (hi + lo rows)
            lraw = st_pool.tile([2 * E, 1], FP32, tag="lraw")
            nc.vector.scalar_tensor_tensor(
                junk_xr, xrT, 1.0, rs_b[0 : 2 * E], AL.mult, AL.mult,
                accum_out=lraw,
            )
            ps_lt = pp_m.tile([1, 2 * E], FP32, tag="pslt")
            nc.tensor.transpose(ps_lt, lraw, ident_sb[0 : 2 * E, 0 : 2 * E])
            lrawT2 = st_pool.tile([1, 2 * E], FP32, tag="lrawT2")
            nc.scalar.copy(lrawT2, ps_lt)
            lrawT = st_pool.tile([1, E], FP32, tag="lrawT")
            nc.vector.tensor_tensor(
                lrawT, lrawT2[:, 0:E], lrawT2[:, E : 2 * E], AL.add
            )
            # sigma = sum_t mu_t * rs_t  (per-partition partial, then all-reduce)
            sg = st_pool.tile([P, 1], FP32, tag="sg")
            nc.vector.tensor_reduce(sg, mrs, axis=mybir.AxisListType.X, op=AL.add)
            sgr = st_pool.tile([P, 1], FP32, tag="sgr")
            nc.gpsimd.partition_all_reduce(sgr, sg, P, bass_isa.ReduceOp.add)
            # logits = lrawT - sigma*u + c   (ucbc[:,0] = -u, ucbc[:,1] = c)
            logits = st_pool.tile([1, E], FP32, tag="logits")
            nc.vector.scalar_tensor_tensor(
                logits, ucbc[0:1, 0], sgr[0:1], lrawT, AL.mult, AL.add
            )
            nc.vector.tensor_tensor(logits, logits, ucbc[0:1, 1], AL.add)

            # ---- gating (single partition, then broadcast) ----
            lmax = st_pool.tile([1, 1], FP32, tag="lmax")
            nc.vector.tensor_reduce(lmax, logits, axis=mybir.AxisListType.X, op=AL.max)
            u0 = st_pool.tile([1, E], FP32, tag="u0")
            nc.vector.tensor_scalar(u0, logits, lmax, None, AL.subtract)
            ex = st_pool.tile([1, E], FP32, tag="ex")
            nc.vector.tensor_scalar(ex, u0, 0.25, 1.0, AL.mult, AL.add)
            for coef in (3.0, 2.0, 1.0):
                nc.vector.tensor_mul(ex, ex, u0)
                nc.vector.tensor_scalar(ex, ex, 1.0 / coef, 1.0, AL.mult, AL.add)
            denom = st_pool.tile([1, 1], FP32, tag="denom")
            nc.vector.tensor_reduce(denom, ex, axis=mybir.AxisListType.X, op=AL.add)
            oh = st_pool.tile([1, E], FP32, tag="oh")
            nc.vector.tensor_scalar(oh, logits, lmax, None, AL.is_equal)
            # gidx = [gate, idx*128, idx*80] on one partition, then broadcast
            idxf = st_pool.tile([1, 1], FP32, tag="idxf")
            nc.vector.scalar_tensor_tensor(
                junk8[0:1], oh, 1.0, iota_sb[0:1], AL.mult, AL.mult,
                accum_out=idxf,
            )
            gidx = st_pool.tile([1, 4], FP32, tag="gidx")
            nc.vector.reciprocal(gidx[:, 0:1], denom)
            nc.vector.tensor_scalar_mul(gidx[:, 1:2], idxf, float(P))
            nc.vector.tensor_scalar_mul(gidx[:, 2:3], idxf, float(NWU))
            nc.vector.tensor_scalar_mul(gidx[:, 3:4], idxf, float(H))
            gidx_b = st_pool.tile([P, 4], FP32, tag="gidxb")
            nc.gpsimd.partition_broadcast(gidx_b, gidx)
            gate = gidx_b[:, 0:1]
            wd_idxs = st_pool.tile([P, P // 16], I16, tag="wdidx")
            nc.vector.tensor_scalar_add(wd_idxs, wdb_sb, gidx_b[:, 1:2])
            wu_idxs = st_pool.tile([P, NWU // 16], I16, tag="wuidx")
            nc.vector.tensor_scalar_add(wu_idxs, wub_sb, gidx_b[:, 2:3])
            h_idxs = st_pool.tile([P, H // 16], I16, tag="hidx")
            nc.vector.tensor_scalar_add(h_idxs, hb_sb, gidx_b[:, 3:4])
            wd_eff = w_pool.tile([P, 1, DC * H], BF16, tag="wd")
            nc.gpsimd.dma_gather(wd_eff, wdg_ext, wd_idxs, P, P, DC * H)
            wu_g = w_pool.tile([P, 1, D], BF16, tag="wu")
            nc.gpsimd.dma_gather(wu_g, wug_ext, wu_idxs, NWU, NWU, D)

            # ---- b_down / -colsum(Wd) selection via gather ----
            sel_g = w_pool.tile([P, 1, H], FP32, tag="selg")
            nc.gpsimd.dma_gather(sel_g, bdncs_ext, h_idxs, H, H, H)
            sel2 = sel_g[0:H, 0]  # [H, H]; col 0 = bd, col 1 = -cs

            state[b] = (xt, rs_b, mrs_b, wd_eff, wu_g, sel2, gate)

        def back(b):
            xt, rs_b, mrs_b, wd_eff, wu_g, sel2, gate = state.pop(b)
            wu_eff = wu_g[0:HP, 0]  # [HP, D]

            # ---- mm1 + fused-LN affine + relu -> hT [HP, S] bf16 ----
            hT = h_pool.tile([HP, S], BF16, tag="hT")
            nc.vector.memset(hT[H:HP], 1.0)
            for half in range(2):
                sl = slice(half * 512, (half + 1) * 512)
                ps_h = pp_h.tile([H, 512], FP32, tag="psh")
                for dc in range(DC):
                    nc.tensor.matmul(
                        ps_h,
                        wd_eff[:, 0, dc * H : (dc + 1) * H],
                        xt[:, dc, sl],
                        start=(dc == 0),
                        stop=(dc == DC - 1),
                    )
                e1 = e_pool.tile([H, 512], FP32, tag="e1")
                nc.vector.tensor_tensor(e1, ps_h, rs_b[0:H, sl], AL.mult)
                e2 = e_pool.tile([H, 512], FP32, tag="e2")
                nc.vector.scalar_tensor_tensor(
                    e2, mrs_b[:, sl], sel2[:, 1:2], e1, AL.mult, AL.add
                )
                nc.scalar.activation(
                    hT[0:H, sl], e2, mybir.ActivationFunctionType.Relu,
                    bias=sel2[:, 0:1],
                )

            # ---- mm2; gate applied in the PSUM->SBUF copy ----
            y_sb = y_pool.tile([P, TC, D], BF16, tag="y")
            for t in range(TC):
                for half in range(2):
                    sl = slice(half * 512, (half + 1) * 512)
                    ps_y = pp_y.tile([P, 512], FP32)
                    nc.tensor.matmul(
                        ps_y,
                        hT[:, t * P : (t + 1) * P],
                        wu_eff[:, sl],
                        start=True,
                        stop=True,
                    )
                    nc.scalar.mul(y_sb[:, t, sl], ps_y, gate)
            nc.sync.dma_start(
                out_ext[b].rearrange("(tc p) d -> p tc d", p=P), y_sb
            )

        front(0)
        front(1)
        for b in range(BLOC):
            if b + 2 < BLOC:
                front(b + 2)
            back(b)


def _fold_weights(inputs):
    g = np.asarray(inputs["ln_g"], np.float32)
    bb = np.asarray(inputs["ln_b"], np.float32)
    bn_g = np.asarray(inputs["bn_g"], np.float32)
    bn_b = np.asarray(inputs["bn_b"], np.float32)
    bn_mean = np.asarray(inputs["bn_mean"], np.float32)
    bn_var = np.asarray(inputs["bn_var"], np.float32)
    Wr = np.asarray(inputs["Wr"], np.float32)
    br = np.asarray(inputs["br"], np.float32)
    W_down = np.asarray(inputs["W_down"], np.float32)
    b_down = np.asarray(inputs["b_down"], np.float32)
    W_up = np.asarray(inputs["W_up"], np.float32)
    b_up = np.asarray(inputs["b_up"], np.float32)

    q = 1.0 / np.sqrt(bn_var + np.float32(EPS))
    wr_f = ((g * q * bn_g / np.float32(S))[:, None] * Wr).astype(np.float32)
    c = (((bb - bn_mean) * q * bn_g + bn_b) @ Wr + br).astype(np.float32)
    u = wr_f.sum(axis=0)  # [E]
    ucbc = np.stack([-u, c], axis=0)  # [2, E]
    ucbc = np.ascontiguousarray(
        np.broadcast_to(ucbc[None], (P, 2, E)).astype(np.float32)
    )

    wd_f = (g[None, :, None] * W_down).astype(ml_dtypes.bfloat16)  # [E, D, H]
    cs = wd_f.astype(np.float32).sum(axis=1)  # [E, H] colsums of the bf16 weights
    bd_f = (b_down + np.einsum("d,edh->eh", bb, W_down)).astype(np.float32)
    bdncs = np.zeros((E, H, H), dtype=np.float32)
    bdncs[:, :, 0] = bd_f
    bdncs[:, :, 1] = -cs
    bdncs = bdncs.reshape(E * H, H)
    wu_f = np.concatenate([W_up, b_up[:, None, :]], axis=1).astype(
        ml_dtypes.bfloat16
    )  # [E, HP, D]

    # gather tables: row (e*P + p) of wdg pairs xt partition p with wd cols
    wdg = (
        wd_f.reshape(E, DC, P, H).transpose(0, 2, 1, 3).reshape(E * P, DC * H)
    )
    wug = np.zeros((E, NWU, D), dtype=ml_dtypes.bfloat16)
    wug[:, :HP] = wu_f
    wug = wug.reshape(E * NWU, D)

    # gather ucode reads index i from slot [16 + i%16, i//16]; mirror into
    # partitions 0..15 too for the simulator's interpretation
    def _idx_table(n):
        t = np.zeros((P, n // 16), dtype=np.int16)
        t[:16, :] = np.arange(n, dtype=np.int16).reshape(n // 16, 16).T
        t[16:32, :] = t[:16, :]
        return t

    wdb = _idx_table(P)
    wub = _idx_table(NWU)
    hb = _idx_table(H)
    iota = np.ascontiguousarray(
        np.broadcast_to(np.arange(E, dtype=np.float32)[None], (P, E))
    )

    wr_hi = wr_f.astype(ml_dtypes.bfloat16)
    wr_lo = (wr_f - wr_hi.astype(np.float32)).astype(ml_dtypes.bfloat16)
    wr2 = np.concatenate([wr_hi, wr_lo], axis=1)  # [D, 2E]

    return {
        "wr": np.ascontiguousarray(wr2.reshape(DC, P, 2 * E)),
        "ucbc": ucbc,
        "wdg": np.ascontiguousarray(wdg),
        "wug": np.ascontiguousarray(wug),
        "bdncs": np.ascontiguousarray(bdncs),
        "ident": np.eye(P, dtype=np.float32),
        "iota": iota,
        "wdb": np.ascontiguousarray(wdb),
        "wub": np.ascontiguousarray(wub),
        "hb": np.ascontiguousarray(hb),
    }


def make_in_maps(inputs):
    params = _fold_weights(inputs)
    x = np.asarray(inputs["x"], np.float32)
    x_bf = x.astype(ml_dtypes.bfloat16)
    in_maps = []
    for i in range(NCORES):
        m = dict(params)
        xb = x_bf[i * BLOC : (i + 1) * BLOC]  # [BLOC, S, D]
        m["xa"] = np.ascontiguousarray(xb)
        m["xt"] = np.ascontiguousarray(
            xb.transpose(0, 2, 1).reshape(BLOC, DC, P, S)
        )
        in_maps.append(m)
    return in_maps


def get_nc():
    if "nc" not in _CACHE:
        _CACHE["nc"] = _build_kernel()
    return _CACHE["nc"]


def kernel(**inputs) -> np.ndarray:
    nc = get_nc()
    in_maps = make_in_maps(inputs)
    res = run_bass_kernel_spmd(nc, in_maps, core_ids=list(range(NCORES)))
    _CACHE["last_result"] = res
    out = np.concatenate(
        [
            np.asarray(res.results[i]["out"]).astype(np.float32)
            for i in range(NCORES)
        ],
        axis=0,
    )
    return out


if __name__ == "__main__":
    nc = get_nc()
    print("build + compile OK")

